# revision 4
# baseline (speedup 1.0000x reference)
"""Trainium2 Bass kernel for EuclideanSimilarity:
out[i, j] = -||z_anc[i] - z_pos_neg[j]||_2
          = -sqrt(a2[i] + b2[j] - 2 * z_anc[i] . z_pos_neg[j])

Sharding: z_anc rows split across 8 cores (1024 rows each); z_pos_neg
replicated.  Each core computes a [1024, 8192] slab of the output.

Per-core device program:
  - ab via TensorE: bf16 matmuls, lhsT = aT block [128d x 128m] stationary,
    rhs = bT [128d x 512n] moving, K = D = 128 (single accumulation step).
  - -0.5*b2 folded into the same PSUM via a rank-2 bf16 matmul
    (lhsT = ones [2 x 128], rhs = [hi;lo] split of -0.5*b2 for precision).
  - a2 folded in as the ScalarE activation per-partition bias:
      ACT: sqrt(-2 * psum + a2) -> fp16   (one pass, evacuates PSUM)
  - DVE negate (fp16 4x mode), DMA out.
"""

import os
import sys

import numpy as np
import ml_dtypes

try:
    import concourse  # noqa: F401
except ImportError:
    for _p in ("/opt/trn_rl_repo", os.path.expanduser("~/.axon_site/_ro/trn_rl_repo")):
        if os.path.isdir(_p) and _p not in sys.path:
            sys.path.insert(0, _p)

import concourse.bass as bass
import concourse.mybir as mybir
import concourse.tile as tile
from concourse import bacc
from concourse import bass_utils

N_CORES = 8
N, M, D = 8192, 8192, 128
R = N // N_CORES  # 1024 rows of z_anc per core
P = 128           # partitions
BANK = 512        # fp32 columns per PSUM bank
GRP = 2048        # columns handled per ACT/DVE/DMA group (4 banks)
MT = R // P       # 8 m-tiles per core
NG = M // GRP     # 4 groups per m-tile row

OUT_DT = mybir.dt.float16
OUT_NP = np.float16

_BF16 = ml_dtypes.bfloat16

_nc_cache = None


def _build():
    f32 = mybir.dt.float32
    bf16 = mybir.dt.bfloat16

    nc = bacc.Bacc("TRN2", debug=False, target_bir_lowering=False)
    aT = nc.dram_tensor("aT", [P, R], bf16, kind="ExternalInput").ap()
    bT = nc.dram_tensor("bT", [P, M], bf16, kind="ExternalInput").ap()
    out = nc.dram_tensor("out", [R, M], OUT_DT, kind="ExternalOutput").ap()

    with tile.TileContext(nc) as tc:
        with tc.tile_pool(name="consts", bufs=1) as consts:
            aT_sb = consts.tile([P, R], bf16)
            nc.sync.dma_start(out=aT_sb, in_=aT)
            bT_sb = consts.tile([P, M], bf16)
            nc.sync.dma_start(out=bT_sb, in_=bT)

            neghalf = consts.tile([P, 1], bf16)
            nc.vector.memset(neghalf, -0.5)
            onescol = consts.tile([P, 1], bf16)
            nc.vector.memset(onescol, 1.0)
            ones2 = consts.tile([2, P], bf16)
            nc.vector.memset(ones2, 1.0)

            b2hl = consts.tile([2, M], bf16)   # hi/lo split of -0.5*b2
            lorow = consts.tile([1, M], bf16)  # lo staging (partition 0)
            a2c = consts.tile([P, MT], f32)    # a2 bias column per m-tile
            asq = consts.tile([P, R], bf16)

            # ---- prologue: b2 and a2 row norms --------------------------
            with (
                tc.tile_pool(name="sq", bufs=3) as sq_pool,
                tc.tile_pool(name="pp", bufs=2, space="PSUM") as pp,
            ):
                for j in range(M // BANK):
                    sl = slice(j * BANK, (j + 1) * BANK)
                    sq = sq_pool.tile([P, BANK], bf16)
                    nc.vector.tensor_mul(sq, bT_sb[:, sl], bT_sb[:, sl])
                    pb = pp.tile([1, BANK], f32, tag="pb")
                    nc.tensor.matmul(pb, lhsT=neghalf, rhs=sq, start=True, stop=True)
                    # hi/lo split: hi = bf16(x); lo = bf16(x - hi)
                    nc.vector.tensor_copy(b2hl[0:1, sl], pb)
                    nc.vector.tensor_sub(lorow[0:1, sl], pb, b2hl[0:1, sl])
                # compute engines cannot write at partition base 1; DMA can
                nc.sync.dma_start(out=b2hl[1:2, :], in_=lorow[0:1, :])

                for j in range(R // BANK):
                    sl = slice(j * BANK, (j + 1) * BANK)
                    nc.vector.tensor_mul(asq[:, sl], aT_sb[:, sl], aT_sb[:, sl])
                for t in range(MT):
                    pa = pp.tile([P, 1], f32, tag="pa")
                    nc.tensor.matmul(
                        pa, lhsT=asq[:, t * P:(t + 1) * P], rhs=onescol,
                        start=True, stop=True,
                    )
                    nc.vector.tensor_copy(a2c[:, t:t + 1], pa)

            # ---- main loop ---------------------------------------------
            with (
                tc.tile_pool(name="mm", bufs=2, space="PSUM") as mm_pool,
                tc.tile_pool(name="o", bufs=4) as o_pool,
                tc.tile_pool(name="on", bufs=4) as on_pool,
            ):
                for t in range(MT):
                    for g in range(NG):
                        ps = mm_pool.tile([P, GRP], f32)
                        for j in range(GRP // BANK):
                            c0 = g * GRP + j * BANK
                            nc.tensor.matmul(
                                ps[:, j * BANK:(j + 1) * BANK],
                                lhsT=aT_sb[:, t * P:(t + 1) * P],
                                rhs=bT_sb[:, c0:c0 + BANK],
                                start=True, stop=False,
                            )
                        for j in range(GRP // BANK):
                            c0 = g * GRP + j * BANK
                            nc.tensor.matmul(
                                ps[:, j * BANK:(j + 1) * BANK],
                                lhsT=ones2,
                                rhs=b2hl[:, c0:c0 + BANK],
                                start=False, stop=True,
                            )
                        o = o_pool.tile([P, GRP], OUT_DT)
                        nc.scalar.activation(
                            o, ps, mybir.ActivationFunctionType.Sqrt,
                            bias=a2c[:, t:t + 1], scale=-2.0,
                        )
                        on = on_pool.tile([P, GRP], OUT_DT)
                        nc.vector.tensor_scalar_mul(on, o, -1.0)
                        nc.sync.dma_start(
                            out=out[t * P:(t + 1) * P, g * GRP:(g + 1) * GRP],
                            in_=on,
                        )

    nc.compile()
    return nc


def _get_nc():
    global _nc_cache
    if _nc_cache is None:
        _nc_cache = _build()
    return _nc_cache


def _in_maps(z_anc, z_pos_neg):
    zaT = np.ascontiguousarray(np.asarray(z_anc, dtype=np.float32).T)
    zbT = np.ascontiguousarray(np.asarray(z_pos_neg, dtype=np.float32).T)
    bT = zbT.astype(_BF16)
    maps = []
    for c in range(N_CORES):
        aT = np.ascontiguousarray(zaT[:, c * R:(c + 1) * R]).astype(_BF16)
        maps.append({"aT": aT, "bT": bT})
    return maps


def run(z_anc, z_pos_neg, **kwargs):
    """Run on hardware; returns (full_output, BassKernelResults)."""
    nc = _get_nc()
    res = bass_utils.run_bass_kernel_spmd(
        nc, _in_maps(z_anc, z_pos_neg), core_ids=list(range(N_CORES)), **kwargs
    )
    out = np.concatenate([r["out"] for r in res.results], axis=0)
    return out.astype(np.float32), res


def kernel(z_anc, z_pos_neg):
    out, _ = run(z_anc, z_pos_neg)
    return out


# revision 6
# speedup vs baseline: 1.9092x; 1.9092x over previous
"""Trainium2 Bass kernel for EuclideanSimilarity:
out[i, j] = -||z_anc[i] - z_pos_neg[j]||_2
          = -sqrt(a2[i] + b2[j] - 2 * z_anc[i] . z_pos_neg[j])

Sharding: z_anc rows split across 8 cores (1024 rows each); z_pos_neg
replicated.  Each core computes a [1024, 8192] slab of the output.

Per-core device program (engine balance: PE ~60us, ACT ~64us, DVE ~40us,
DMA ~48us):
  - ab via TensorE: bf16 K=128 matmuls, lhsT = aT block [128d x 128m]
    stationary, rhs = bT [128d x 512n] moving.
  - b2 folded into the same PSUM accumulation via a SECOND K=128 matmul:
    lhsT = ones [128,128], rhs = b2rep where b2rep[d,n] = (128-b2[n])/256
    replicated across partitions (sum over 128 rows -> (128-b2[n])/2).
    All-K=128 keeps the PE at its ~220ns/MM pipelined cadence (K=2
    rank-1 updates measured 2x slower due to LDW/pipe interaction).
  - b2rep built JIT per n-group: DVE squares -> wred(-1/256 ones) matmul
    column-sum -> DVE +0.5 evac to bf16.  PSUM: 3x2-bank main tiles +
    2x1-bank reduce tiles = 8 banks exactly.
  - a2+128 folded in as the ScalarE activation per-partition bias:
      ACT: sqrt(-2*psum + bias) -> fp16   (one pass, evacuates PSUM)
  - DVE negate (fp16 4x mode), DMA out [128, 1024] fp16 tiles.
"""

import os
import sys

import numpy as np
import ml_dtypes

try:
    import concourse  # noqa: F401
except ImportError:
    for _p in ("/opt/trn_rl_repo", os.path.expanduser("~/.axon_site/_ro/trn_rl_repo")):
        if os.path.isdir(_p) and _p not in sys.path:
            sys.path.insert(0, _p)

import concourse.bass as bass  # noqa: F401
import concourse.mybir as mybir
import concourse.tile as tile
from concourse import bacc
from concourse import bass_utils

N_CORES = 8
N, M, D = 8192, 8192, 128
R = N // N_CORES  # 1024 rows of z_anc per core
P = 128           # partitions
BANK = 512        # fp32 columns per PSUM bank
GRP = 1024        # columns per ACT/DVE/DMA group (2 banks)
MT = R // P       # 8 m-tiles per core
NG = M // GRP     # 8 n-groups

OUT_DT = mybir.dt.float16
_BF16 = ml_dtypes.bfloat16

_nc_cache = None


def _build():
    f32 = mybir.dt.float32
    bf16 = mybir.dt.bfloat16

    nc = bacc.Bacc("TRN2", debug=False, target_bir_lowering=False)
    aT = nc.dram_tensor("aT", [P, R], bf16, kind="ExternalInput").ap()
    bT = nc.dram_tensor("bT", [P, M], bf16, kind="ExternalInput").ap()
    out = nc.dram_tensor("out", [R, M], OUT_DT, kind="ExternalOutput").ap()

    with tile.TileContext(nc) as tc:
        with tc.tile_pool(name="consts", bufs=1) as consts:
            aT_sb = consts.tile([P, R], bf16)
            nc.sync.dma_start(out=aT_sb, in_=aT)
            bT_sb = consts.tile([P, M], bf16)
            # chunked load so dependent work starts early
            for g in range(4):
                sl = slice(g * 2048, (g + 1) * 2048)
                nc.sync.dma_start(out=bT_sb[:, sl], in_=bT[:, sl])

            ones128 = consts.tile([P, P], bf16)
            nc.vector.memset(ones128, 1.0)
            onescol = consts.tile([P, 1], bf16)
            nc.vector.memset(onescol, 1.0)
            wred = consts.tile([P, P], bf16)   # -1/256 for b2 column sums
            nc.vector.memset(wred, -1.0 / 256.0)

            b2rep = consts.tile([P, M], bf16)  # (128 - b2[n]) / 256 broadcast
            asq = consts.tile([P, R], bf16)
            a2c = consts.tile([P, MT], f32)
            a2f = consts.tile([P, MT], f32)    # a2 + 128 (ACT bias columns)

            with (
                tc.tile_pool(name="mm", bufs=3, space="PSUM") as mm_pool,
                tc.tile_pool(name="rp", bufs=2, space="PSUM") as rp_pool,
                tc.tile_pool(name="sq", bufs=3) as sq_pool,
                tc.tile_pool(name="o", bufs=4) as o_pool,
                tc.tile_pool(name="on", bufs=4) as on_pool,
            ):
                # ---- a2: per-row norms of this core's z_anc slice ------
                for j in range(R // BANK):
                    sl = slice(j * BANK, (j + 1) * BANK)
                    nc.vector.tensor_mul(asq[:, sl], aT_sb[:, sl], aT_sb[:, sl])
                for t in range(MT):
                    pa = rp_pool.tile([P, BANK], f32, tag="pr")
                    nc.tensor.matmul(
                        pa[:, 0:1], lhsT=asq[:, t * P:(t + 1) * P], rhs=onescol,
                        start=True, stop=True,
                    )
                    nc.vector.tensor_copy(a2c[:, t:t + 1], pa[:, 0:1])
                nc.vector.tensor_scalar_add(a2f, a2c, 128.0)

                # ---- main loop (n-group-major, b2rep built JIT) --------
                for g in range(NG):
                    # b2rep chunks for this group's columns
                    for j in range(GRP // BANK):
                        c0 = g * GRP + j * BANK
                        sl = slice(c0, c0 + BANK)
                        sq = sq_pool.tile([P, BANK], bf16, tag="sq")
                        nc.vector.tensor_mul(sq, bT_sb[:, sl], bT_sb[:, sl])
                        pr = rp_pool.tile([P, BANK], f32, tag="pr")
                        nc.tensor.matmul(pr, lhsT=wred, rhs=sq,
                                         start=True, stop=True)
                        # b2rep = 0.5 + (-b2/256)
                        nc.vector.tensor_scalar_add(b2rep[:, sl], pr, 0.5)

                    for t in range(MT):
                        ps = mm_pool.tile([P, GRP], f32, tag="ps")
                        for j in range(GRP // BANK):
                            c0 = g * GRP + j * BANK
                            nc.tensor.matmul(
                                ps[:, j * BANK:(j + 1) * BANK],
                                lhsT=aT_sb[:, t * P:(t + 1) * P],
                                rhs=bT_sb[:, c0:c0 + BANK],
                                start=True, stop=False,
                            )
                        for j in range(GRP // BANK):
                            c0 = g * GRP + j * BANK
                            nc.tensor.matmul(
                                ps[:, j * BANK:(j + 1) * BANK],
                                lhsT=ones128,
                                rhs=b2rep[:, c0:c0 + BANK],
                                start=False, stop=True,
                            )
                        o = o_pool.tile([P, GRP], OUT_DT, tag="o")
                        nc.scalar.activation(
                            o, ps, mybir.ActivationFunctionType.Sqrt,
                            bias=a2f[:, t:t + 1], scale=-2.0,
                        )
                        on = on_pool.tile([P, GRP], OUT_DT, tag="on")
                        nc.vector.tensor_scalar_mul(on, o, -1.0)
                        nc.sync.dma_start(
                            out=out[t * P:(t + 1) * P, g * GRP:(g + 1) * GRP],
                            in_=on,
                        )

    nc.compile()
    return nc


def _get_nc():
    global _nc_cache
    if _nc_cache is None:
        _nc_cache = _build()
    return _nc_cache


def _in_maps(z_anc, z_pos_neg):
    zaT = np.ascontiguousarray(np.asarray(z_anc, dtype=np.float32).T)
    zbT = np.ascontiguousarray(np.asarray(z_pos_neg, dtype=np.float32).T)
    bT = zbT.astype(_BF16)
    maps = []
    for c in range(N_CORES):
        aTc = np.ascontiguousarray(zaT[:, c * R:(c + 1) * R]).astype(_BF16)
        maps.append({"aT": aTc, "bT": bT})
    return maps


def run(z_anc, z_pos_neg, **kwargs):
    """Run on hardware; returns (full_output, BassKernelResults)."""
    nc = _get_nc()
    res = bass_utils.run_bass_kernel_spmd(
        nc, _in_maps(z_anc, z_pos_neg), core_ids=list(range(N_CORES)), **kwargs
    )
    out = np.concatenate([r["out"] for r in res.results], axis=0)
    return out.astype(np.float32), res


def kernel(z_anc, z_pos_neg):
    out, _ = run(z_anc, z_pos_neg)
    return out
